# revision 15
# baseline (speedup 1.0000x reference)
"""Trainium2 Bass kernel for NoopSegmenter segment-mean reduction.

Math (per batch row):
  bnd[t]   = (in_boundary[t] != 0), with bnd[0] forced to 1     t in [0, 501)
  cs[l]    = sum_{t<=l} bnd[t]                                  l in [0, 500)
  nB       = sum_t bnd[t]
  M0[s, l] = (cs[l] == s+1)                                     s in [0, 50)
  len[s]   = sum_l M0[s, l]
  valid[s] = (nB >= s+2)
  w[s]     = valid[s] / max(len[s], 1)
  out[s,:] = w[s] * sum_l M0[s, l] * x[l, :]
  msum[s]  = len[s] * w[s]
  (in_boundary is passed through unchanged as the third output)

Device strategy: pure data parallel, 8 batches per core on 8 cores.
cs is computed with small triangular matmuls over the (transposed)
boundary vector; M0^T is built by comparing cs (per-partition scalar)
against an iota row; the segment reduction is a PE matmul with M0^T as
the stationary operand.  x is split host-side into bf16 hi/lo planes
(same total bytes as fp32) so the PE runs at full bf16 rate with ~1e-5
relative error; products with the 0/1 mask are exact and accumulate in
fp32 PSUM.
"""

import contextlib

import numpy as np
import ml_dtypes

import concourse.bacc as bacc
import concourse.tile as tile
from concourse import mybir
from concourse.bass_utils import run_bass_kernel_spmd

B, L, D = 64, 500, 1024
S = 50
NBS = L + 1      # boundary slots
LP = 512         # padded frame/slot count
NCORES = 8
BPC = B // NCORES  # batches per core
NCH = LP // 128    # 128-partition chunks of the (padded) frame axis

F32 = mybir.dt.float32
BF16 = mybir.dt.bfloat16


def build_nc():
    nc = bacc.Bacc(None, target_bir_lowering=False)

    xs = nc.dram_tensor("xs", [BPC, 2, 128, NCH * D], BF16, kind="ExternalInput")
    ibt = nc.dram_tensor("ibt", [128, NCH, BPC], F32, kind="ExternalInput")
    tri = nc.dram_tensor("tri", [128, NCH, LP], BF16, kind="ExternalInput")
    iotaf = nc.dram_tensor("iotaf", [128, S], F32, kind="ExternalInput")
    iota2 = nc.dram_tensor("iota2", [1, S], F32, kind="ExternalInput")
    out = nc.dram_tensor("out", [BPC, S, D], F32, kind="ExternalOutput")
    msum = nc.dram_tensor("msum", [1, BPC * S], F32, kind="ExternalOutput")

    xs_ap, ibt_ap, tri_ap = xs.ap(), ibt.ap(), tri.ap()
    iotaf_ap, iota2_ap = iotaf.ap(), iota2.ap()
    out_ap, msum_ap = out.ap(), msum.ap()

    eq = mybir.AluOpType.is_equal
    le = mybir.AluOpType.is_le
    ne = mybir.AluOpType.not_equal

    with tile.TileContext(nc) as tc:
        with (
            tc.tile_pool(name="consts", bufs=1) as consts,
            tc.tile_pool(name="xp", bufs=10) as xp,
            tc.tile_pool(name="mp", bufs=8) as mp,
            tc.tile_pool(name="sp", bufs=4) as sp,
            tc.tile_pool(name="outp", bufs=3) as outp,
        ):
            tri_sb = consts.tile([128, NCH, LP], BF16)
            nc.sync.dma_start(out=tri_sb, in_=tri_ap)
            iotaf_sb = consts.tile([128, S], F32)
            nc.sync.dma_start(out=iotaf_sb, in_=iotaf_ap)
            iota2_sb = consts.tile([1, S], F32)
            nc.sync.dma_start(out=iota2_sb, in_=iota2_ap)
            ibt_sb = consts.tile([128, NCH, BPC], F32)
            nc.sync.dma_start(out=ibt_sb, in_=ibt_ap)

            ones_sb = consts.tile([128, 1], BF16)
            nc.vector.memset(ones_sb, 1.0)
            id1_sb = consts.tile([1, 1], F32)
            nc.vector.memset(id1_sb, 1.0)

            # bnd^T in bf16 (0/1), slot 0 forced to 1, padded slots stay 0
            bndt_sb = consts.tile([128, NCH, BPC], BF16)
            for c in range(NCH):
                nc.vector.tensor_scalar(
                    out=bndt_sb[:, c, :], in0=ibt_sb[:, c, :],
                    scalar1=0.0, scalar2=None, op0=ne,
                )
            nc.vector.memset(bndt_sb[0:1, 0, :], 1.0)

            cst_sb = consts.tile([128, NCH, BPC], F32)  # cs^T, [l, b]
            nb_sb = consts.tile([1, BPC], F32)          # total boundary count
            msum_sb = consts.tile([1, BPC * S], F32)

            # cs^T via triangular matmuls; tri[:, t-chunk, l] zeroes l>=500
            with tc.tile_pool(name="pcs", bufs=1, space="PSUM") as pcs:
                for lc in range(NCH):
                    cs_ps = pcs.tile([128, BPC], F32, bufs=4)
                    for t in range(lc + 1):
                        nc.tensor.matmul(
                            cs_ps,
                            lhsT=tri_sb[:, t, lc * 128:(lc + 1) * 128],
                            rhs=bndt_sb[:, t, :],
                            start=(t == 0), stop=(t == lc),
                        )
                    nc.vector.tensor_copy(out=cst_sb[:, lc, :], in_=cs_ps)
                nb_ps = pcs.tile([1, BPC], F32, bufs=1)
                for t in range(NCH):
                    nc.tensor.matmul(
                        nb_ps, lhsT=ones_sb, rhs=bndt_sb[:, t, :],
                        start=(t == 0), stop=(t == NCH - 1),
                    )
                nc.vector.tensor_copy(out=nb_sb, in_=nb_ps)

            stack = contextlib.ExitStack()
            pmain = stack.enter_context(tc.tile_pool(name="pmain", bufs=4, space="PSUM"))
            plen = stack.enter_context(tc.tile_pool(name="plen", bufs=2, space="PSUM"))
            pw = stack.enter_context(tc.tile_pool(name="pw", bufs=2, space="PSUM"))

            for pair in range(BPC // 2):
                bpair = (2 * pair, 2 * pair + 1)
                xts, m0s = [], []
                for b in bpair:
                    xt_h = xp.tile([128, NCH * D], BF16, tag="xt")
                    nc.sync.dma_start(out=xt_h, in_=xs_ap[b, 0])
                    xt_l = xp.tile([128, NCH * D], BF16, tag="xt")
                    nc.sync.dma_start(out=xt_l, in_=xs_ap[b, 1])
                    xts.append((xt_h, xt_l))
                    m0 = []
                    for c in range(NCH):
                        m = mp.tile([128, S], BF16, tag="m0")
                        nc.vector.tensor_scalar(
                            out=m, in0=iotaf_sb,
                            scalar1=cst_sb[:, c, b:b + 1], scalar2=None, op0=eq,
                        )
                        m0.append(m)
                    m0s.append(m0)

                # both batches' w packed in one row: cols [0:50] and [64:114]
                w_row2 = sp.tile([1, 128], F32, tag="w_row2")
                for bi, b in enumerate(bpair):
                    off = 64 * bi
                    len_ps = plen.tile([1, S], F32)
                    for c in range(NCH):
                        nc.tensor.matmul(
                            len_ps, lhsT=ones_sb, rhs=m0s[bi][c],
                            start=(c == 0), stop=(c == NCH - 1),
                        )
                    valid = sp.tile([1, S], F32, tag="valid")
                    nc.vector.tensor_scalar(
                        out=valid, in0=iota2_sb,
                        scalar1=nb_sb[0:1, b:b + 1], scalar2=None, op0=le,
                    )
                    lmax = sp.tile([1, S], F32, tag="lmax")
                    nc.vector.tensor_scalar_max(out=lmax, in0=len_ps, scalar1=1.0)
                    rec = sp.tile([1, S], F32, tag="rec")
                    nc.vector.reciprocal(out=rec, in_=lmax)
                    nc.vector.tensor_mul(
                        out=w_row2[0:1, off:off + S], in0=rec, in1=valid
                    )
                    nc.vector.tensor_mul(
                        out=msum_sb[0:1, b * S:(b + 1) * S],
                        in0=len_ps, in1=w_row2[0:1, off:off + S],
                    )
                wt_ps = pw.tile([128, 1], F32)
                nc.tensor.transpose(out=wt_ps, in_=w_row2, identity=id1_sb)
                w_pair = sp.tile([128, 1], F32, tag="w_pair")
                nc.vector.tensor_copy(out=w_pair, in_=wt_ps)

                out_pair = outp.tile([128, D], F32)
                for n in range(2):
                    mm_ps = pmain.tile([128, 512], F32)
                    k = 0
                    for h in range(2):
                        for c in range(NCH):
                            sl = slice(c * D + n * 512, c * D + n * 512 + 512)
                            nc.tensor.matmul(
                                mm_ps[0:S, :], lhsT=m0s[0][c], rhs=xts[0][h][:, sl],
                                start=(k == 0), stop=(k == 7),
                                tile_position=(0, 0), skip_group_check=True,
                            )
                            nc.tensor.matmul(
                                mm_ps[64:64 + S, :], lhsT=m0s[1][c], rhs=xts[1][h][:, sl],
                                start=(k == 0), stop=(k == 7),
                                tile_position=(0, 64), skip_group_check=True,
                            )
                            k += 1
                    nc.scalar.activation(
                        out=out_pair[0:S, n * 512:(n + 1) * 512], in_=mm_ps[0:S, :],
                        func=mybir.ActivationFunctionType.Copy, scale=w_pair[0:S],
                    )
                    nc.scalar.activation(
                        out=out_pair[64:64 + S, n * 512:(n + 1) * 512],
                        in_=mm_ps[64:64 + S, :],
                        func=mybir.ActivationFunctionType.Copy,
                        scale=w_pair[64:64 + S],
                    )
                nc.sync.dma_start(out=out_ap[bpair[0]], in_=out_pair[0:S, :])
                nc.sync.dma_start(out=out_ap[bpair[1]], in_=out_pair[64:64 + S, :])

            nc.sync.dma_start(out=msum_ap, in_=msum_sb)
            stack.close()

    nc.compile()
    return nc


_NC_CACHE = {}


def get_nc():
    if "nc" not in _NC_CACHE:
        _NC_CACHE["nc"] = build_nc()
    return _NC_CACHE["nc"]


def make_in_maps(x, in_boundary):
    x = np.ascontiguousarray(x, dtype=np.float32)
    ib = np.ascontiguousarray(in_boundary, dtype=np.float32)

    hi = x.astype(ml_dtypes.bfloat16)
    lo = (x - hi.astype(np.float32)).astype(ml_dtypes.bfloat16)

    # [B, 512, D] padded, then partition-first [B, 128, NCH*D]
    def pack(plane):
        p = np.zeros((B, LP, D), dtype=ml_dtypes.bfloat16)
        p[:, :L, :] = plane
        return p.reshape(B, NCH, 128, D).transpose(0, 2, 1, 3).reshape(B, 128, NCH * D)

    hi_p, lo_p = pack(hi), pack(lo)
    xs_all = np.stack([hi_p, lo_p], axis=1)  # [B, 2, 128, NCH*D]

    ibt_all = np.zeros((B, LP), dtype=np.float32)
    ibt_all[:, :NBS] = ib
    # per core -> [128, NCH, BPC]

    t_idx = np.arange(LP)[:, None]
    l_idx = np.arange(LP)[None, :]
    tri = ((t_idx <= l_idx) & (l_idx < L)).astype(ml_dtypes.bfloat16)  # [512, 512]
    tri_packed = np.ascontiguousarray(
        tri.reshape(NCH, 128, LP).transpose(1, 0, 2)
    )  # [128, NCH, LP]

    iotaf = np.tile(np.arange(1, S + 1, dtype=np.float32), (128, 1))
    iota2 = np.arange(2, S + 2, dtype=np.float32)[None, :]

    in_maps = []
    for c in range(NCORES):
        sl = slice(c * BPC, (c + 1) * BPC)
        ibt_c = np.ascontiguousarray(
            ibt_all[sl].T.reshape(NCH, 128, BPC).transpose(1, 0, 2)
        )  # [128, NCH, BPC]
        in_maps.append({
            "xs": np.ascontiguousarray(xs_all[sl]),
            "ibt": ibt_c,
            "tri": tri_packed,
            "iotaf": iotaf,
            "iota2": iota2,
        })
    return in_maps


def gather(results):
    out = np.concatenate([r["out"] for r in results], axis=0)  # [B, S, D]
    msum = np.concatenate(
        [r["msum"].reshape(BPC, S) for r in results], axis=0
    )  # [B, S]
    return out.astype(np.float32), msum.astype(np.float32)


def kernel(x, in_boundary):
    nc = get_nc()
    in_maps = make_in_maps(x, in_boundary)
    res = run_bass_kernel_spmd(nc, in_maps, core_ids=list(range(NCORES)))
    out, msum = gather(res.results)
    return out, msum, np.asarray(in_boundary)


# revision 16
# speedup vs baseline: 1.1561x; 1.1561x over previous
"""Trainium2 Bass kernel for NoopSegmenter segment-mean reduction.

Math (per batch row):
  bnd[t]   = (in_boundary[t] != 0), with bnd[0] forced to 1     t in [0, 501)
  cs[l]    = sum_{t<=l} bnd[t]                                  l in [0, 500)
  nB       = sum_t bnd[t]
  M0[s, l] = (cs[l] == s+1)                                     s in [0, 50)
  len[s]   = sum_l M0[s, l]
  valid[s] = (nB >= s+2)
  w[s]     = valid[s] / max(len[s], 1)
  out[s,:] = w[s] * sum_l M0[s, l] * x[l, :]
  msum[s]  = len[s] * w[s]
  (in_boundary is passed through unchanged as the third output)

Device strategy: pure data parallel, 8 batches per core on 8 cores.
cs is computed with small triangular matmuls over the (transposed)
boundary vector; M0^T is built by comparing cs (per-partition scalar)
against an iota row; the segment reduction is a PE matmul with M0^T as
the stationary operand.  x is split host-side into bf16 hi/lo planes
(same total bytes as fp32) so the PE runs at full bf16 rate with ~1e-5
relative error; products with the 0/1 mask are exact and accumulate in
fp32 PSUM.
"""

import contextlib

import numpy as np
import ml_dtypes

import concourse.bacc as bacc
import concourse.tile as tile
from concourse import mybir
from concourse.bass_utils import run_bass_kernel_spmd

B, L, D = 64, 500, 1024
S = 50
NBS = L + 1      # boundary slots
LP = 512         # padded frame/slot count
NCORES = 8
BPC = B // NCORES  # batches per core
NCH = LP // 128    # 128-partition chunks of the (padded) frame axis

F32 = mybir.dt.float32
BF16 = mybir.dt.bfloat16
FP8 = mybir.dt.float8e4


def build_nc():
    nc = bacc.Bacc(None, target_bir_lowering=False)

    xhi = nc.dram_tensor("xhi", [BPC, 128, NCH * D], BF16, kind="ExternalInput")
    xlo = nc.dram_tensor("xlo", [BPC, 128, NCH * D], FP8, kind="ExternalInput")
    ibt = nc.dram_tensor("ibt", [128, NCH, BPC], F32, kind="ExternalInput")
    tri = nc.dram_tensor("tri", [128, NCH, LP], FP8, kind="ExternalInput")
    iotaf = nc.dram_tensor("iotaf", [128, S], F32, kind="ExternalInput")
    iota2 = nc.dram_tensor("iota2", [1, S], F32, kind="ExternalInput")
    out = nc.dram_tensor("out", [BPC, S, D], F32, kind="ExternalOutput")
    msum = nc.dram_tensor("msum", [1, BPC * S], F32, kind="ExternalOutput")

    xhi_ap, xlo_ap, ibt_ap, tri_ap = xhi.ap(), xlo.ap(), ibt.ap(), tri.ap()
    iotaf_ap, iota2_ap = iotaf.ap(), iota2.ap()
    out_ap, msum_ap = out.ap(), msum.ap()

    eq = mybir.AluOpType.is_equal
    le = mybir.AluOpType.is_le
    ne = mybir.AluOpType.not_equal

    with tile.TileContext(nc) as tc:
        with (
            tc.tile_pool(name="consts", bufs=1) as consts,
            tc.tile_pool(name="xp", bufs=10) as xp,
            tc.tile_pool(name="mp", bufs=8) as mp,
            tc.tile_pool(name="sp", bufs=4) as sp,
            tc.tile_pool(name="outp", bufs=3) as outp,
        ):
            tri_sb = consts.tile([128, NCH, LP], FP8)
            nc.sync.dma_start(out=tri_sb, in_=tri_ap)
            iotaf_sb = consts.tile([128, S], F32)
            nc.sync.dma_start(out=iotaf_sb, in_=iotaf_ap)
            iota2_sb = consts.tile([1, S], F32)
            nc.sync.dma_start(out=iota2_sb, in_=iota2_ap)
            ibt_sb = consts.tile([128, NCH, BPC], F32)
            nc.sync.dma_start(out=ibt_sb, in_=ibt_ap)

            ones_sb = consts.tile([128, 1], BF16)
            nc.vector.memset(ones_sb, 1.0)
            ones8_sb = consts.tile([128, 1], FP8)
            nc.vector.memset(ones8_sb, 1.0)
            id1_sb = consts.tile([1, 1], F32)
            nc.vector.memset(id1_sb, 1.0)

            # bnd^T in bf16 (0/1), slot 0 forced to 1, padded slots stay 0
            bndt_sb = consts.tile([128, NCH, BPC], FP8)
            for c in range(NCH):
                nc.vector.tensor_scalar(
                    out=bndt_sb[:, c, :], in0=ibt_sb[:, c, :],
                    scalar1=0.0, scalar2=None, op0=ne,
                )
            nc.vector.memset(bndt_sb[0:1, 0, :], 1.0)

            cst_sb = consts.tile([128, NCH, BPC], F32)  # cs^T, [l, b]
            nb_sb = consts.tile([1, BPC], F32)          # total boundary count
            msum_sb = consts.tile([1, BPC * S], F32)

            # cs^T via triangular matmuls; tri[:, t-chunk, l] zeroes l>=500
            with tc.tile_pool(name="pcs", bufs=1, space="PSUM") as pcs:
                for lc in range(NCH):
                    cs_ps = pcs.tile([128, BPC], F32, bufs=4)
                    for t in range(lc + 1):
                        nc.tensor.matmul(
                            cs_ps,
                            lhsT=tri_sb[:, t, lc * 128:(lc + 1) * 128],
                            rhs=bndt_sb[:, t, :],
                            start=(t == 0), stop=(t == lc),
                        )
                    nc.vector.tensor_copy(out=cst_sb[:, lc, :], in_=cs_ps)
                nb_ps = pcs.tile([1, BPC], F32, bufs=1)
                for t in range(NCH):
                    nc.tensor.matmul(
                        nb_ps, lhsT=ones8_sb, rhs=bndt_sb[:, t, :],
                        start=(t == 0), stop=(t == NCH - 1),
                    )
                nc.vector.tensor_copy(out=nb_sb, in_=nb_ps)

            stack = contextlib.ExitStack()
            pmain = stack.enter_context(tc.tile_pool(name="pmain", bufs=4, space="PSUM"))
            plen = stack.enter_context(tc.tile_pool(name="plen", bufs=2, space="PSUM"))
            pw = stack.enter_context(tc.tile_pool(name="pw", bufs=2, space="PSUM"))

            for pair in range(BPC // 2):
                bpair = (2 * pair, 2 * pair + 1)
                xts, m0s = [], []
                for b in bpair:
                    xt_h = xp.tile([128, NCH * D], BF16, tag="xh")
                    nc.sync.dma_start(out=xt_h, in_=xhi_ap[b])
                    xt_l = xp.tile([128, NCH * D], FP8, tag="xl")
                    nc.sync.dma_start(out=xt_l, in_=xlo_ap[b])
                    xts.append((xt_h, xt_l))
                    m0 = []
                    for c in range(NCH):
                        mh = mp.tile([128, S], BF16, tag="m0h")
                        nc.vector.tensor_scalar(
                            out=mh, in0=iotaf_sb,
                            scalar1=cst_sb[:, c, b:b + 1], scalar2=None, op0=eq,
                        )
                        ml = mp.tile([128, S], FP8, tag="m0l")
                        nc.vector.tensor_scalar(
                            out=ml, in0=iotaf_sb,
                            scalar1=cst_sb[:, c, b:b + 1], scalar2=0.0625,
                            op0=eq, op1=mybir.AluOpType.mult,
                        )
                        m0.append((mh, ml))
                    m0s.append(m0)

                # both batches' w packed in one row: cols [0:50] and [64:114]
                w_row2 = sp.tile([1, 128], F32, tag="w_row2")
                for bi, b in enumerate(bpair):
                    off = 64 * bi
                    len_ps = plen.tile([1, S], F32)
                    for c in range(NCH):
                        nc.tensor.matmul(
                            len_ps, lhsT=ones_sb, rhs=m0s[bi][c][0],
                            start=(c == 0), stop=(c == NCH - 1),
                        )
                    valid = sp.tile([1, S], F32, tag="valid")
                    nc.vector.tensor_scalar(
                        out=valid, in0=iota2_sb,
                        scalar1=nb_sb[0:1, b:b + 1], scalar2=None, op0=le,
                    )
                    lmax = sp.tile([1, S], F32, tag="lmax")
                    nc.vector.tensor_scalar_max(out=lmax, in0=len_ps, scalar1=1.0)
                    rec = sp.tile([1, S], F32, tag="rec")
                    nc.vector.reciprocal(out=rec, in_=lmax)
                    nc.vector.tensor_mul(
                        out=w_row2[0:1, off:off + S], in0=rec, in1=valid
                    )
                    nc.vector.tensor_mul(
                        out=msum_sb[0:1, b * S:(b + 1) * S],
                        in0=len_ps, in1=w_row2[0:1, off:off + S],
                    )
                wt_ps = pw.tile([128, 1], F32)
                nc.tensor.transpose(out=wt_ps, in_=w_row2, identity=id1_sb)
                w_pair = sp.tile([128, 1], F32, tag="w_pair")
                nc.vector.tensor_copy(out=w_pair, in_=wt_ps)

                out_pair = outp.tile([128, D], F32)
                for n in range(2):
                    mm_ps = pmain.tile([128, 512], F32)
                    k = 0
                    for h in range(2):
                        for c in range(NCH):
                            sl = slice(c * D + n * 512, c * D + n * 512 + 512)
                            nc.tensor.matmul(
                                mm_ps[0:S, :], lhsT=m0s[0][c][h], rhs=xts[0][h][:, sl],
                                start=(k == 0), stop=(k == 7),
                                tile_position=(0, 0), skip_group_check=True,
                            )
                            nc.tensor.matmul(
                                mm_ps[64:64 + S, :], lhsT=m0s[1][c][h], rhs=xts[1][h][:, sl],
                                start=(k == 0), stop=(k == 7),
                                tile_position=(0, 64), skip_group_check=True,
                            )
                            k += 1
                    nc.scalar.activation(
                        out=out_pair[0:S, n * 512:(n + 1) * 512], in_=mm_ps[0:S, :],
                        func=mybir.ActivationFunctionType.Copy, scale=w_pair[0:S],
                    )
                    nc.scalar.activation(
                        out=out_pair[64:64 + S, n * 512:(n + 1) * 512],
                        in_=mm_ps[64:64 + S, :],
                        func=mybir.ActivationFunctionType.Copy,
                        scale=w_pair[64:64 + S],
                    )
                nc.sync.dma_start(out=out_ap[bpair[0]], in_=out_pair[0:S, :])
                nc.sync.dma_start(out=out_ap[bpair[1]], in_=out_pair[64:64 + S, :])

            nc.sync.dma_start(out=msum_ap, in_=msum_sb)
            stack.close()

    nc.compile()
    return nc


_NC_CACHE = {}


def get_nc():
    if "nc" not in _NC_CACHE:
        _NC_CACHE["nc"] = build_nc()
    return _NC_CACHE["nc"]


def make_in_maps(x, in_boundary):
    x = np.ascontiguousarray(x, dtype=np.float32)
    ib = np.ascontiguousarray(in_boundary, dtype=np.float32)

    hi = x.astype(ml_dtypes.bfloat16)
    lo = ((x - hi.astype(np.float32)) * 16.0).astype(ml_dtypes.float8_e4m3)

    # [B, 512, D] padded, then partition-first [B, 128, NCH*D]
    def pack(plane, dt):
        p = np.zeros((B, LP, D), dtype=dt)
        p[:, :L, :] = plane
        return p.reshape(B, NCH, 128, D).transpose(0, 2, 1, 3).reshape(B, 128, NCH * D)

    hi_p = pack(hi, ml_dtypes.bfloat16)
    lo_p = pack(lo, ml_dtypes.float8_e4m3)

    ibt_all = np.zeros((B, LP), dtype=np.float32)
    ibt_all[:, :NBS] = ib
    # per core -> [128, NCH, BPC]

    t_idx = np.arange(LP)[:, None]
    l_idx = np.arange(LP)[None, :]
    tri = ((t_idx <= l_idx) & (l_idx < L)).astype(ml_dtypes.float8_e4m3)  # [512, 512]
    tri_packed = np.ascontiguousarray(
        tri.reshape(NCH, 128, LP).transpose(1, 0, 2)
    )  # [128, NCH, LP]

    iotaf = np.tile(np.arange(1, S + 1, dtype=np.float32), (128, 1))
    iota2 = np.arange(2, S + 2, dtype=np.float32)[None, :]

    in_maps = []
    for c in range(NCORES):
        sl = slice(c * BPC, (c + 1) * BPC)
        ibt_c = np.ascontiguousarray(
            ibt_all[sl].T.reshape(NCH, 128, BPC).transpose(1, 0, 2)
        )  # [128, NCH, BPC]
        in_maps.append({
            "xhi": np.ascontiguousarray(hi_p[sl]),
            "xlo": np.ascontiguousarray(lo_p[sl]),
            "ibt": ibt_c,
            "tri": tri_packed,
            "iotaf": iotaf,
            "iota2": iota2,
        })
    return in_maps


def gather(results):
    out = np.concatenate([r["out"] for r in results], axis=0)  # [B, S, D]
    msum = np.concatenate(
        [r["msum"].reshape(BPC, S) for r in results], axis=0
    )  # [B, S]
    return out.astype(np.float32), msum.astype(np.float32)


def kernel(x, in_boundary):
    nc = get_nc()
    in_maps = make_in_maps(x, in_boundary)
    res = run_bass_kernel_spmd(nc, in_maps, core_ids=list(range(NCORES)))
    out, msum = gather(res.results)
    return out, msum, np.asarray(in_boundary)


# revision 17
# speedup vs baseline: 1.2638x; 1.0931x over previous
"""Trainium2 Bass kernel for NoopSegmenter segment-mean reduction.

Math (per batch row):
  bnd[t]   = (in_boundary[t] != 0), with bnd[0] forced to 1     t in [0, 501)
  cs[l]    = sum_{t<=l} bnd[t]                                  l in [0, 500)
  nB       = sum_t bnd[t]
  M0[s, l] = (cs[l] == s+1)                                     s in [0, 50)
  len[s]   = sum_l M0[s, l]
  valid[s] = (nB >= s+2)
  w[s]     = valid[s] / max(len[s], 1)
  out[s,:] = w[s] * sum_l M0[s, l] * x[l, :]
  msum[s]  = len[s] * w[s]
  (in_boundary is passed through unchanged as the third output)

Device strategy: pure data parallel, 8 batches per core on 8 cores.
cs is computed with small triangular matmuls over the (transposed)
boundary vector; M0^T is built by comparing cs (per-partition scalar)
against an iota row; the segment reduction is a PE matmul with M0^T as
the stationary operand.  x is split host-side into bf16 hi/lo planes
(same total bytes as fp32) so the PE runs at full bf16 rate with ~1e-5
relative error; products with the 0/1 mask are exact and accumulate in
fp32 PSUM.
"""

import contextlib

import numpy as np
import ml_dtypes

import concourse.bacc as bacc
import concourse.tile as tile
from concourse import mybir
from concourse.bass_utils import run_bass_kernel_spmd

B, L, D = 64, 500, 1024
S = 50
NBS = L + 1      # boundary slots
LP = 512         # padded frame/slot count
NCORES = 8
BPC = B // NCORES  # batches per core
NCH = LP // 128    # 128-partition chunks of the (padded) frame axis

F32 = mybir.dt.float32
BF16 = mybir.dt.bfloat16
FP8 = mybir.dt.float8e4


def build_nc():
    nc = bacc.Bacc(None, target_bir_lowering=False)

    xhi = nc.dram_tensor("xhi", [BPC, 128, NCH * D], BF16, kind="ExternalInput")
    xlo = nc.dram_tensor("xlo", [BPC, 128, NCH * D], FP8, kind="ExternalInput")
    ibt = nc.dram_tensor("ibt", [128, NCH, BPC], F32, kind="ExternalInput")
    tri = nc.dram_tensor("tri", [128, NCH, LP], FP8, kind="ExternalInput")
    iotaf = nc.dram_tensor("iotaf", [128, S], F32, kind="ExternalInput")
    iota2 = nc.dram_tensor("iota2", [1, S], F32, kind="ExternalInput")
    out = nc.dram_tensor("out", [BPC, S, D], F32, kind="ExternalOutput")
    msum = nc.dram_tensor("msum", [1, BPC * S], F32, kind="ExternalOutput")

    xhi_ap, xlo_ap, ibt_ap, tri_ap = xhi.ap(), xlo.ap(), ibt.ap(), tri.ap()
    iotaf_ap, iota2_ap = iotaf.ap(), iota2.ap()
    out_ap, msum_ap = out.ap(), msum.ap()

    eq = mybir.AluOpType.is_equal
    le = mybir.AluOpType.is_le
    ne = mybir.AluOpType.not_equal

    with tile.TileContext(nc) as tc:
        with (
            tc.tile_pool(name="consts", bufs=1) as consts,
            tc.tile_pool(name="xp", bufs=10) as xp,
            tc.tile_pool(name="mp", bufs=8) as mp,
            tc.tile_pool(name="sp", bufs=4) as sp,
            tc.tile_pool(name="outp", bufs=3) as outp,
        ):
            tri_sb = consts.tile([128, NCH, LP], FP8)
            nc.sync.dma_start(out=tri_sb, in_=tri_ap)
            iotaf_sb = consts.tile([128, S], F32)
            nc.sync.dma_start(out=iotaf_sb, in_=iotaf_ap)
            iota2_sb = consts.tile([1, S], F32)
            nc.sync.dma_start(out=iota2_sb, in_=iota2_ap)
            ibt_sb = consts.tile([128, NCH, BPC], F32)
            nc.sync.dma_start(out=ibt_sb, in_=ibt_ap)

            ones_sb = consts.tile([128, 1], BF16)
            nc.vector.memset(ones_sb, 1.0)
            ones8_sb = consts.tile([128, 1], FP8)
            nc.vector.memset(ones8_sb, 1.0)
            id1_sb = consts.tile([1, 1], F32)
            nc.vector.memset(id1_sb, 1.0)

            # bnd^T in bf16 (0/1), slot 0 forced to 1, padded slots stay 0
            bndt_sb = consts.tile([128, NCH, BPC], FP8)
            for c in range(NCH):
                nc.vector.tensor_scalar(
                    out=bndt_sb[:, c, :], in0=ibt_sb[:, c, :],
                    scalar1=0.0, scalar2=None, op0=ne,
                )
            nc.vector.memset(bndt_sb[0:1, 0, :], 1.0)

            cst_sb = consts.tile([128, NCH, BPC], F32)  # cs^T, [l, b]
            nb_sb = consts.tile([1, BPC], F32)          # total boundary count
            msum_sb = consts.tile([1, BPC * S], F32)

            # cs^T via triangular matmuls; tri[:, t-chunk, l] zeroes l>=500
            with tc.tile_pool(name="pcs", bufs=1, space="PSUM") as pcs:
                for lc in range(NCH):
                    cs_ps = pcs.tile([128, BPC], F32, bufs=4)
                    for t in range(lc + 1):
                        nc.tensor.matmul(
                            cs_ps,
                            lhsT=tri_sb[:, t, lc * 128:(lc + 1) * 128],
                            rhs=bndt_sb[:, t, :],
                            start=(t == 0), stop=(t == lc),
                        )
                    nc.vector.tensor_copy(out=cst_sb[:, lc, :], in_=cs_ps)
                nb_ps = pcs.tile([1, BPC], F32, bufs=1)
                for t in range(NCH):
                    nc.tensor.matmul(
                        nb_ps, lhsT=ones8_sb, rhs=bndt_sb[:, t, :],
                        start=(t == 0), stop=(t == NCH - 1),
                    )
                nc.vector.tensor_copy(out=nb_sb, in_=nb_ps)

            stack = contextlib.ExitStack()
            pmain = stack.enter_context(tc.tile_pool(name="pmain", bufs=4, space="PSUM"))
            plen = stack.enter_context(tc.tile_pool(name="plen", bufs=2, space="PSUM"))
            pw = stack.enter_context(tc.tile_pool(name="pw", bufs=2, space="PSUM"))

            for pair in range(BPC // 2):
                bpair = (2 * pair, 2 * pair + 1)
                xts, m0s = [], []
                for b in bpair:
                    xt_h = xp.tile([128, NCH * D], BF16, tag="xh")
                    nc.sync.dma_start(out=xt_h, in_=xhi_ap[b])
                    xt_l = xp.tile([128, NCH * D], FP8, tag="xl")
                    nc.sync.dma_start(out=xt_l, in_=xlo_ap[b])
                    xts.append((xt_h, xt_l))
                    m0 = []
                    for c in range(NCH):
                        mh = mp.tile([128, S], BF16, tag="m0h")
                        nc.vector.tensor_scalar(
                            out=mh, in0=iotaf_sb,
                            scalar1=cst_sb[:, c, b:b + 1], scalar2=None, op0=eq,
                        )
                        ml = mp.tile([128, S], FP8, tag="m0l")
                        nc.vector.tensor_scalar(
                            out=ml, in0=iotaf_sb,
                            scalar1=cst_sb[:, c, b:b + 1], scalar2=0.0625,
                            op0=eq, op1=mybir.AluOpType.mult,
                        )
                        m0.append((mh, ml))
                    m0s.append(m0)

                # both batches' w packed in one row: cols [0:50] and [64:114]
                w_row2 = sp.tile([1, 128], F32, tag="w_row2")
                for bi, b in enumerate(bpair):
                    off = 64 * bi
                    len_ps = plen.tile([1, S], F32)
                    for c in range(NCH):
                        nc.tensor.matmul(
                            len_ps, lhsT=ones_sb, rhs=m0s[bi][c][0],
                            start=(c == 0), stop=(c == NCH - 1),
                        )
                    valid = sp.tile([1, S], F32, tag="valid")
                    nc.vector.tensor_scalar(
                        out=valid, in0=iota2_sb,
                        scalar1=nb_sb[0:1, b:b + 1], scalar2=None, op0=le,
                    )
                    lmax = sp.tile([1, S], F32, tag="lmax")
                    nc.vector.tensor_scalar_max(out=lmax, in0=len_ps, scalar1=1.0)
                    rec = sp.tile([1, S], F32, tag="rec")
                    nc.vector.reciprocal(out=rec, in_=lmax)
                    nc.vector.tensor_mul(
                        out=w_row2[0:1, off:off + S], in0=rec, in1=valid
                    )
                    nc.vector.tensor_mul(
                        out=msum_sb[0:1, b * S:(b + 1) * S],
                        in0=len_ps, in1=w_row2[0:1, off:off + S],
                    )
                wt_ps = pw.tile([128, 1], F32)
                nc.tensor.transpose(out=wt_ps, in_=w_row2, identity=id1_sb)
                w_pair = sp.tile([128, 1], F32, tag="w_pair")
                nc.vector.tensor_copy(out=w_pair, in_=wt_ps)

                out_pair = outp.tile([128, D], F32)
                for n in range(2):
                    mm_ps = pmain.tile([128, 512], F32)
                    k = 0
                    for h in range(2):
                        for c in range(NCH):
                            sl = slice(c * D + n * 512, c * D + n * 512 + 512)
                            nc.tensor.matmul(
                                mm_ps[0:S, :], lhsT=m0s[0][c][h], rhs=xts[0][h][:, sl],
                                start=(k == 0), stop=(k == 7),
                                tile_position=(0, 0), skip_group_check=True,
                            )
                            nc.tensor.matmul(
                                mm_ps[64:64 + S, :], lhsT=m0s[1][c][h], rhs=xts[1][h][:, sl],
                                start=(k == 0), stop=(k == 7),
                                tile_position=(0, 64), skip_group_check=True,
                            )
                            k += 1
                    nc.scalar.activation(
                        out=out_pair[0:S, n * 512:(n + 1) * 512], in_=mm_ps[0:S, :],
                        func=mybir.ActivationFunctionType.Copy, scale=w_pair[0:S],
                    )
                    nc.scalar.activation(
                        out=out_pair[64:64 + S, n * 512:(n + 1) * 512],
                        in_=mm_ps[64:64 + S, :],
                        func=mybir.ActivationFunctionType.Copy,
                        scale=w_pair[64:64 + S],
                    )
                nc.scalar.dma_start(out=out_ap[bpair[0]], in_=out_pair[0:S, :])
                nc.scalar.dma_start(out=out_ap[bpair[1]], in_=out_pair[64:64 + S, :])

            nc.scalar.dma_start(out=msum_ap, in_=msum_sb)
            stack.close()

    nc.compile()
    return nc


_NC_CACHE = {}


def get_nc():
    if "nc" not in _NC_CACHE:
        _NC_CACHE["nc"] = build_nc()
    return _NC_CACHE["nc"]


def make_in_maps(x, in_boundary):
    x = np.ascontiguousarray(x, dtype=np.float32)
    ib = np.ascontiguousarray(in_boundary, dtype=np.float32)

    hi = x.astype(ml_dtypes.bfloat16)
    lo = ((x - hi.astype(np.float32)) * 16.0).astype(ml_dtypes.float8_e4m3)

    # [B, 512, D] padded, then partition-first [B, 128, NCH*D]
    def pack(plane, dt):
        p = np.zeros((B, LP, D), dtype=dt)
        p[:, :L, :] = plane
        return p.reshape(B, NCH, 128, D).transpose(0, 2, 1, 3).reshape(B, 128, NCH * D)

    hi_p = pack(hi, ml_dtypes.bfloat16)
    lo_p = pack(lo, ml_dtypes.float8_e4m3)

    ibt_all = np.zeros((B, LP), dtype=np.float32)
    ibt_all[:, :NBS] = ib
    # per core -> [128, NCH, BPC]

    t_idx = np.arange(LP)[:, None]
    l_idx = np.arange(LP)[None, :]
    tri = ((t_idx <= l_idx) & (l_idx < L)).astype(ml_dtypes.float8_e4m3)  # [512, 512]
    tri_packed = np.ascontiguousarray(
        tri.reshape(NCH, 128, LP).transpose(1, 0, 2)
    )  # [128, NCH, LP]

    iotaf = np.tile(np.arange(1, S + 1, dtype=np.float32), (128, 1))
    iota2 = np.arange(2, S + 2, dtype=np.float32)[None, :]

    in_maps = []
    for c in range(NCORES):
        sl = slice(c * BPC, (c + 1) * BPC)
        ibt_c = np.ascontiguousarray(
            ibt_all[sl].T.reshape(NCH, 128, BPC).transpose(1, 0, 2)
        )  # [128, NCH, BPC]
        in_maps.append({
            "xhi": np.ascontiguousarray(hi_p[sl]),
            "xlo": np.ascontiguousarray(lo_p[sl]),
            "ibt": ibt_c,
            "tri": tri_packed,
            "iotaf": iotaf,
            "iota2": iota2,
        })
    return in_maps


def gather(results):
    out = np.concatenate([r["out"] for r in results], axis=0)  # [B, S, D]
    msum = np.concatenate(
        [r["msum"].reshape(BPC, S) for r in results], axis=0
    )  # [B, S]
    return out.astype(np.float32), msum.astype(np.float32)


def kernel(x, in_boundary):
    nc = get_nc()
    in_maps = make_in_maps(x, in_boundary)
    res = run_bass_kernel_spmd(nc, in_maps, core_ids=list(range(NCORES)))
    out, msum = gather(res.results)
    return out, msum, np.asarray(in_boundary)


# revision 18
# speedup vs baseline: 1.3734x; 1.0867x over previous
"""Trainium2 Bass kernel for NoopSegmenter segment-mean reduction.

Math (per batch row):
  bnd[t]   = (in_boundary[t] != 0), with bnd[0] forced to 1     t in [0, 501)
  cs[l]    = sum_{t<=l} bnd[t]                                  l in [0, 500)
  nB       = sum_t bnd[t]
  M0[s, l] = (cs[l] == s+1)                                     s in [0, 50)
  len[s]   = sum_l M0[s, l]
  valid[s] = (nB >= s+2)
  w[s]     = valid[s] / max(len[s], 1)
  out[s,:] = w[s] * sum_l M0[s, l] * x[l, :]
  msum[s]  = len[s] * w[s]
  (in_boundary is passed through unchanged as the third output)

Device strategy: pure data parallel, 8 batches per core on 8 cores.
cs is computed with small triangular matmuls over the (transposed)
boundary vector; M0^T is built by comparing cs (per-partition scalar)
against an iota row; the segment reduction is a PE matmul with M0^T as
the stationary operand.  x is split host-side into bf16 hi/lo planes
(same total bytes as fp32) so the PE runs at full bf16 rate with ~1e-5
relative error; products with the 0/1 mask are exact and accumulate in
fp32 PSUM.
"""

import contextlib

import numpy as np
import ml_dtypes

import concourse.bacc as bacc
import concourse.tile as tile
from concourse import mybir
from concourse.bass_utils import run_bass_kernel_spmd

B, L, D = 64, 500, 1024
S = 50
NBS = L + 1      # boundary slots
LP = 512         # padded frame/slot count
NCORES = 8
BPC = B // NCORES  # batches per core
NCH = LP // 128    # 128-partition chunks of the (padded) frame axis

F32 = mybir.dt.float32
BF16 = mybir.dt.bfloat16
FP8 = mybir.dt.float8e4
FP16 = mybir.dt.float16


def build_nc():
    nc = bacc.Bacc(None, target_bir_lowering=False)

    xhi = nc.dram_tensor("xhi", [BPC, 128, NCH * D], FP16, kind="ExternalInput")
    xlo = nc.dram_tensor("xlo", [BPC, 128, NCH * D], FP8, kind="ExternalInput")
    ibt = nc.dram_tensor("ibt", [128, NCH, BPC], F32, kind="ExternalInput")
    tri = nc.dram_tensor("tri", [128, NCH, LP], FP8, kind="ExternalInput")
    iotaf = nc.dram_tensor("iotaf", [128, S], F32, kind="ExternalInput")
    iota2 = nc.dram_tensor("iota2", [1, S], F32, kind="ExternalInput")
    out = nc.dram_tensor("out", [BPC, S, D], F32, kind="ExternalOutput")
    msum = nc.dram_tensor("msum", [1, BPC * S], F32, kind="ExternalOutput")

    xhi_ap, xlo_ap, ibt_ap, tri_ap = xhi.ap(), xlo.ap(), ibt.ap(), tri.ap()
    iotaf_ap, iota2_ap = iotaf.ap(), iota2.ap()
    out_ap, msum_ap = out.ap(), msum.ap()

    eq = mybir.AluOpType.is_equal
    le = mybir.AluOpType.is_le
    ne = mybir.AluOpType.not_equal

    with tile.TileContext(nc) as tc:
        with (
            tc.tile_pool(name="consts", bufs=1) as consts,
            tc.tile_pool(name="xp", bufs=10) as xp,
            tc.tile_pool(name="mp", bufs=8) as mp,
            tc.tile_pool(name="sp", bufs=4) as sp,
            tc.tile_pool(name="outp", bufs=3) as outp,
        ):
            tri_sb = consts.tile([128, NCH, LP], FP8)
            nc.sync.dma_start(out=tri_sb, in_=tri_ap)
            iotaf_sb = consts.tile([128, S], F32)
            nc.sync.dma_start(out=iotaf_sb, in_=iotaf_ap)
            iota2_sb = consts.tile([1, S], F32)
            nc.sync.dma_start(out=iota2_sb, in_=iota2_ap)
            ibt_sb = consts.tile([128, NCH, BPC], F32)
            nc.sync.dma_start(out=ibt_sb, in_=ibt_ap)

            ones_sb = consts.tile([128, 1], FP16)
            nc.vector.memset(ones_sb, 1.0)
            ones8_sb = consts.tile([128, 1], FP8)
            nc.vector.memset(ones8_sb, 1.0)
            id1_sb = consts.tile([1, 1], F32)
            nc.vector.memset(id1_sb, 1.0)

            # bnd^T in bf16 (0/1), slot 0 forced to 1, padded slots stay 0
            bndt_sb = consts.tile([128, NCH, BPC], FP8)
            for c in range(NCH):
                nc.vector.tensor_scalar(
                    out=bndt_sb[:, c, :], in0=ibt_sb[:, c, :],
                    scalar1=0.0, scalar2=None, op0=ne,
                )
            nc.vector.memset(bndt_sb[0:1, 0, :], 1.0)

            cst_sb = consts.tile([128, NCH, BPC], F32)  # cs^T, [l, b]
            nb_sb = consts.tile([1, BPC], F32)          # total boundary count
            msum_sb = consts.tile([1, BPC * S], F32)

            # cs^T via triangular matmuls; tri[:, t-chunk, l] zeroes l>=500
            with tc.tile_pool(name="pcs", bufs=1, space="PSUM") as pcs:
                for lc in range(NCH):
                    cs_ps = pcs.tile([128, BPC], F32, bufs=4)
                    for t in range(lc + 1):
                        nc.tensor.matmul(
                            cs_ps,
                            lhsT=tri_sb[:, t, lc * 128:(lc + 1) * 128],
                            rhs=bndt_sb[:, t, :],
                            start=(t == 0), stop=(t == lc),
                        )
                    nc.vector.tensor_copy(out=cst_sb[:, lc, :], in_=cs_ps)
                nb_ps = pcs.tile([1, BPC], F32, bufs=1)
                for t in range(NCH):
                    nc.tensor.matmul(
                        nb_ps, lhsT=ones8_sb, rhs=bndt_sb[:, t, :],
                        start=(t == 0), stop=(t == NCH - 1),
                    )
                nc.vector.tensor_copy(out=nb_sb, in_=nb_ps)

            stack = contextlib.ExitStack()
            pmain = stack.enter_context(tc.tile_pool(name="pmain", bufs=4, space="PSUM"))
            plen = stack.enter_context(tc.tile_pool(name="plen", bufs=2, space="PSUM"))
            pw = stack.enter_context(tc.tile_pool(name="pw", bufs=2, space="PSUM"))

            for pair in range(BPC // 2):
                bpair = (2 * pair, 2 * pair + 1)
                xts, m0s = [], []
                for b in bpair:
                    xt_h = xp.tile([128, NCH * D], FP16, tag="xh")
                    nc.sync.dma_start(out=xt_h, in_=xhi_ap[b])
                    xt_l = xp.tile([128, NCH * D], FP8, tag="xl")
                    nc.sync.dma_start(out=xt_l, in_=xlo_ap[b])
                    xts.append((xt_h, xt_l))
                    m0 = []
                    for c in range(NCH):
                        mh = mp.tile([128, S], FP16, tag="m0h")
                        nc.vector.tensor_scalar(
                            out=mh, in0=iotaf_sb,
                            scalar1=cst_sb[:, c, b:b + 1], scalar2=None, op0=eq,
                        )
                        ml = mp.tile([128, S], FP8, tag="m0l")
                        nc.vector.tensor_scalar(
                            out=ml, in0=iotaf_sb,
                            scalar1=cst_sb[:, c, b:b + 1], scalar2=0.015625,
                            op0=eq, op1=mybir.AluOpType.mult,
                        )
                        m0.append((mh, ml))
                    m0s.append(m0)

                # both batches' w packed in one row: cols [0:50] and [64:114]
                w_row2 = sp.tile([1, 128], F32, tag="w_row2")
                for bi, b in enumerate(bpair):
                    off = 64 * bi
                    len_ps = plen.tile([1, S], F32)
                    for c in range(NCH):
                        nc.tensor.matmul(
                            len_ps, lhsT=ones_sb, rhs=m0s[bi][c][0],
                            start=(c == 0), stop=(c == NCH - 1),
                        )
                    valid = sp.tile([1, S], F32, tag="valid")
                    nc.vector.tensor_scalar(
                        out=valid, in0=iota2_sb,
                        scalar1=nb_sb[0:1, b:b + 1], scalar2=None, op0=le,
                    )
                    lmax = sp.tile([1, S], F32, tag="lmax")
                    nc.vector.tensor_scalar_max(out=lmax, in0=len_ps, scalar1=1.0)
                    rec = sp.tile([1, S], F32, tag="rec")
                    nc.vector.reciprocal(out=rec, in_=lmax)
                    nc.vector.tensor_mul(
                        out=w_row2[0:1, off:off + S], in0=rec, in1=valid
                    )
                    nc.vector.tensor_mul(
                        out=msum_sb[0:1, b * S:(b + 1) * S],
                        in0=len_ps, in1=w_row2[0:1, off:off + S],
                    )
                wt_ps = pw.tile([128, 1], F32)
                nc.tensor.transpose(out=wt_ps, in_=w_row2, identity=id1_sb)
                w_pair = sp.tile([128, 1], F32, tag="w_pair")
                nc.vector.tensor_copy(out=w_pair, in_=wt_ps)

                out_pair = outp.tile([128, D], F32)
                for n in range(2):
                    mm_ps = pmain.tile([128, 512], F32)
                    k = 0
                    for h in range(2):
                        for c in range(NCH):
                            sl = slice(c * D + n * 512, c * D + n * 512 + 512)
                            nc.tensor.matmul(
                                mm_ps[0:S, :], lhsT=m0s[0][c][h], rhs=xts[0][h][:, sl],
                                start=(k == 0), stop=(k == 7),
                                tile_position=(0, 0), skip_group_check=True,
                            )
                            nc.tensor.matmul(
                                mm_ps[64:64 + S, :], lhsT=m0s[1][c][h], rhs=xts[1][h][:, sl],
                                start=(k == 0), stop=(k == 7),
                                tile_position=(0, 64), skip_group_check=True,
                            )
                            k += 1
                    nc.scalar.activation(
                        out=out_pair[0:S, n * 512:(n + 1) * 512], in_=mm_ps[0:S, :],
                        func=mybir.ActivationFunctionType.Copy, scale=w_pair[0:S],
                    )
                    nc.scalar.activation(
                        out=out_pair[64:64 + S, n * 512:(n + 1) * 512],
                        in_=mm_ps[64:64 + S, :],
                        func=mybir.ActivationFunctionType.Copy,
                        scale=w_pair[64:64 + S],
                    )
                nc.scalar.dma_start(out=out_ap[bpair[0]], in_=out_pair[0:S, :])
                nc.scalar.dma_start(out=out_ap[bpair[1]], in_=out_pair[64:64 + S, :])
                nc.scalar.dma_start(
                    out=msum_ap[0:1, bpair[0] * S:(bpair[1] + 1) * S],
                    in_=msum_sb[0:1, bpair[0] * S:(bpair[1] + 1) * S],
                )

            stack.close()

    nc.compile()
    return nc


_NC_CACHE = {}


def get_nc():
    if "nc" not in _NC_CACHE:
        _NC_CACHE["nc"] = build_nc()
    return _NC_CACHE["nc"]


def make_in_maps(x, in_boundary):
    x = np.ascontiguousarray(x, dtype=np.float32)
    ib = np.ascontiguousarray(in_boundary, dtype=np.float32)

    hi = x.astype(np.float16)
    lo = ((x - hi.astype(np.float32)) * 64.0).astype(ml_dtypes.float8_e4m3)

    # [B, 512, D] padded, then partition-first [B, 128, NCH*D]
    def pack(plane, dt):
        p = np.zeros((B, LP, D), dtype=dt)
        p[:, :L, :] = plane
        return p.reshape(B, NCH, 128, D).transpose(0, 2, 1, 3).reshape(B, 128, NCH * D)

    hi_p = pack(hi, np.float16)
    lo_p = pack(lo, ml_dtypes.float8_e4m3)

    ibt_all = np.zeros((B, LP), dtype=np.float32)
    ibt_all[:, :NBS] = ib
    # per core -> [128, NCH, BPC]

    t_idx = np.arange(LP)[:, None]
    l_idx = np.arange(LP)[None, :]
    tri = ((t_idx <= l_idx) & (l_idx < L)).astype(ml_dtypes.float8_e4m3)  # [512, 512]
    tri_packed = np.ascontiguousarray(
        tri.reshape(NCH, 128, LP).transpose(1, 0, 2)
    )  # [128, NCH, LP]

    iotaf = np.tile(np.arange(1, S + 1, dtype=np.float32), (128, 1))
    iota2 = np.arange(2, S + 2, dtype=np.float32)[None, :]

    in_maps = []
    for c in range(NCORES):
        sl = slice(c * BPC, (c + 1) * BPC)
        ibt_c = np.ascontiguousarray(
            ibt_all[sl].T.reshape(NCH, 128, BPC).transpose(1, 0, 2)
        )  # [128, NCH, BPC]
        in_maps.append({
            "xhi": np.ascontiguousarray(hi_p[sl]),
            "xlo": np.ascontiguousarray(lo_p[sl]),
            "ibt": ibt_c,
            "tri": tri_packed,
            "iotaf": iotaf,
            "iota2": iota2,
        })
    return in_maps


def gather(results):
    out = np.concatenate([r["out"] for r in results], axis=0)  # [B, S, D]
    msum = np.concatenate(
        [r["msum"].reshape(BPC, S) for r in results], axis=0
    )  # [B, S]
    return out.astype(np.float32), msum.astype(np.float32)


def kernel(x, in_boundary):
    nc = get_nc()
    in_maps = make_in_maps(x, in_boundary)
    res = run_bass_kernel_spmd(nc, in_maps, core_ids=list(range(NCORES)))
    out, msum = gather(res.results)
    return out, msum, np.asarray(in_boundary)
